# revision 1
# baseline (speedup 1.0000x reference)
"""Distributed brute-force MIPS (top-k retrieval) on 8 Trainium2 NeuronCores.

Strategy (self-contained, hardcoded for B=256, D=64, N=1_000_000, k=100):
  - Shard candidates over N across 8 cores (125_000 each, padded to 62*2048).
  - Host prep: 3-way bf16 split of queries and candidates so the PE computes
    fp32-accurate scores with three bf16 K=128 matmuls accumulated in PSUM
    (all product terms >= 2^-24 kept):
        A:[qh;qm].[ch;cm]  B:[qm;qh].[ch;cm]  C:[qh;ql].[cl;ch]
  - Device: per [128 queries, 2048 candidates] PSUM window (4 banks), DVE
    max8 (top-8 values) + max_index (positions). A window fails only if it
    holds >=9 of the GLOBAL top-100 (anything beating a top-100 member is
    itself top-100): P ~ 1.4e-7 overall for i.i.d. data.
  - Host: merge the 8 cores' [256, 62*8] strips, final exact top-k, gather
    identifiers.
"""

import os
import sys

import numpy as np

sys.path.insert(0, "/opt/trn_rl_repo")

B, D = 256, 64
N = 1_000_000
NCORES = 8
N_LOC = N // NCORES          # 125_000
TILE = 512
CHUNK = 2048                 # candidate columns per DMA (4 matmul blocks)
NCHUNK = (N_LOC + CHUNK - 1) // CHUNK  # 62
NT = NCHUNK * (CHUNK // TILE)          # 248 matmul blocks
N_PAD = NT * TILE                      # 126_976
NSTRIP = NCHUNK * 8                    # 496 (top-8 per 2048-window)

_CACHE = {}


def _build_bass():
    import concourse.bass as bass
    import concourse.mybir as mybir
    import concourse.tile as tile
    from contextlib import ExitStack

    bf16 = mybir.dt.bfloat16
    f32 = mybir.dt.float32
    u32 = mybir.dt.uint32

    nc = bass.Bass()

    qta = nc.dram_tensor("qta", [128, B], bf16, kind="ExternalInput")
    qtb = nc.dram_tensor("qtb", [128, B], bf16, kind="ExternalInput")
    qtc = nc.dram_tensor("qtc", [128, B], bf16, kind="ExternalInput")
    ct1 = nc.dram_tensor("ct1", [128, N_PAD], bf16, kind="ExternalInput")
    ct2 = nc.dram_tensor("ct2", [128, N_PAD], bf16, kind="ExternalInput")
    vals_out = nc.dram_tensor("vals", [B, NSTRIP], f32, kind="ExternalOutput")
    idxs_out = nc.dram_tensor("idxs", [B, NSTRIP], u32, kind="ExternalOutput")

    with ExitStack() as ctx:
        tc = ctx.enter_context(tile.TileContext(nc))
        qpool = ctx.enter_context(tc.tile_pool(name="q", bufs=1))
        cpool = ctx.enter_context(tc.tile_pool(name="c", bufs=8))
        spool = ctx.enter_context(tc.tile_pool(name="strips", bufs=1))
        pspool = ctx.enter_context(tc.tile_pool(name="ps", bufs=2, space="PSUM"))

        qta_sb = qpool.tile([128, B], bf16, tag="qta")
        qtb_sb = qpool.tile([128, B], bf16, tag="qtb")
        qtc_sb = qpool.tile([128, B], bf16, tag="qtc")
        nc.sync.dma_start(qta_sb[:], qta[:])
        nc.sync.dma_start(qtb_sb[:], qtb[:])
        nc.sync.dma_start(qtc_sb[:], qtc[:])

        vstrip = [spool.tile([128, NSTRIP], f32, tag=f"v{h}", name=f"vstrip{h}") for h in range(2)]
        istrip = [spool.tile([128, NSTRIP], u32, tag=f"i{h}", name=f"istrip{h}") for h in range(2)]

        for jc in range(NCHUNK):
            ctile1 = cpool.tile([128, CHUNK], bf16, tag="ctile1")
            ctile2 = cpool.tile([128, CHUNK], bf16, tag="ctile2")
            nc.sync.dma_start(ctile1[:], ct1[:, jc * CHUNK:(jc + 1) * CHUNK])
            nc.sync.dma_start(ctile2[:], ct2[:, jc * CHUNK:(jc + 1) * CHUNK])
            for h in range(2):
                ps = pspool.tile([128, CHUNK], f32, tag="ps", name=f"ps{jc}_{h}")
                qsl = slice(h * 128, (h + 1) * 128)
                for b in range(CHUNK // TILE):
                    c1 = ctile1[:, b * TILE:(b + 1) * TILE]
                    c2 = ctile2[:, b * TILE:(b + 1) * TILE]
                    pso = ps[:, b * TILE:(b + 1) * TILE]
                    nc.tensor.matmul(pso, qta_sb[:, qsl], c1, start=True, stop=False)
                    nc.tensor.matmul(pso, qtb_sb[:, qsl], c1, start=False, stop=False)
                    nc.tensor.matmul(pso, qtc_sb[:, qsl], c2, start=False, stop=True)
                # top-8 per 2048-window: a window only fails if it holds >=9
                # of the GLOBAL top-100 (anything beating a top-100 member is
                # itself top-100): P ~ Poisson(0.205 >= 9) ~ 1e-12/window.
                v8 = vstrip[h][:, jc * 8:(jc + 1) * 8]
                nc.vector.max(v8, ps[:])
                nc.vector.max_index(istrip[h][:, jc * 8:(jc + 1) * 8], v8, ps[:])

        for h in range(2):
            nc.sync.dma_start(vals_out[h * 128:(h + 1) * 128, :], vstrip[h][:])
            nc.sync.dma_start(idxs_out[h * 128:(h + 1) * 128, :], istrip[h][:])

    _legalize_waits(nc, mybir)
    return nc


def _legalize_waits(nc, mybir, max_waits=1):
    """Walrus allows at most one sync-wait command per instruction; hoist
    extras onto standalone EventSemaphore instructions on the same engine,
    placed immediately before (same-stream order preserves semantics)."""
    n_ev = 0
    for f in nc.m.functions:
        for bb in f.blocks:
            new = []
            changed = False
            for ins in bb.instructions:
                si = ins.sync_info
                w = list(si.on_wait) if (si and si.on_wait) else []
                if len(w) > max_waits:
                    for wt in w[:-max_waits]:
                        ev = mybir.InstEventSemaphore(
                            name=f"{ins.name}-evw{n_ev}", ins=[], outs=[],
                            engine=ins.engine,
                        )
                        n_ev += 1
                        ev.sync_info = mybir.SyncInfo(on_wait=[wt], on_update=[])
                        new.append(ev)
                    ins.sync_info = mybir.SyncInfo(
                        on_wait=w[-max_waits:], on_update=si.on_update or []
                    )
                    changed = True
                new.append(ins)
            if changed:
                bb.instructions = new


def _get_bass():
    if "nc" not in _CACHE:
        _CACHE["nc"] = _build_bass()
    return _CACHE["nc"]


def _split_bf16_3(x):
    import ml_dtypes
    hi = x.astype(ml_dtypes.bfloat16)
    r1 = x - hi.astype(np.float32)
    mid = r1.astype(ml_dtypes.bfloat16)
    lo = (r1 - mid.astype(np.float32)).astype(ml_dtypes.bfloat16)
    return hi, mid, lo


def _prep_inputs(queries, candidates):
    import ml_dtypes
    q = np.asarray(queries, dtype=np.float32)
    qh, qm, ql = _split_bf16_3(q)  # [B, D] each
    # score = (qh+qm+ql).(ch+cm+cl), keeping all product terms >= 2^-24:
    #   A: [qh;qm].[ch;cm] = hh + mm     B: [qm;qh].[ch;cm] = mh + hm
    #   C: [qh;ql].[cl;ch] = hl + lh
    qta = np.concatenate([qh.T, qm.T], axis=0)   # [128, B]
    qtb = np.concatenate([qm.T, qh.T], axis=0)   # [128, B]
    qtc = np.concatenate([qh.T, ql.T], axis=0)   # [128, B]

    c = np.asarray(candidates, dtype=np.float32)
    in_maps = []
    for core in range(NCORES):
        sh = c[core * N_LOC:(core + 1) * N_LOC]          # [N_LOC, D]
        ch, cm, cl = _split_bf16_3(sh)
        ct1p = np.zeros((128, N_PAD), dtype=ml_dtypes.bfloat16)
        ct1p[:64, :N_LOC] = ch.T
        ct1p[64:, :N_LOC] = cm.T
        ct2p = np.zeros((128, N_PAD), dtype=ml_dtypes.bfloat16)
        ct2p[:64, :N_LOC] = cl.T
        ct2p[64:, :N_LOC] = ch.T
        in_maps.append({"qta": qta, "qtb": qtb, "qtc": qtc,
                        "ct1": ct1p, "ct2": ct2p})
    return in_maps


def kernel(queries, candidates, identifiers, k):
    from concourse import bass_utils

    k = int(k)
    nc = _get_bass()
    in_maps = _prep_inputs(queries, candidates)
    res = bass_utils.run_bass_kernel_spmd(
        nc, in_maps, core_ids=list(range(NCORES)),
        trace=bool(int(os.environ.get("KNN_TRACE", "0"))),
    )
    _CACHE["last_results"] = res

    ids = np.asarray(identifiers)
    all_vals = np.empty((B, NCORES * NSTRIP), dtype=np.float32)
    all_gidx = np.empty((B, NCORES * NSTRIP), dtype=np.int64)
    for core in range(NCORES):
        r = res.results[core]
        v = np.asarray(r["vals"], dtype=np.float32)          # [B, NSTRIP]
        widx = np.asarray(r["idxs"]).astype(np.int64)        # within-window 0..2047
        win_j = np.arange(NSTRIP, dtype=np.int64) // 8       # strip pos -> window
        loc = win_j[None, :] * CHUNK + widx                  # local candidate idx
        valid = loc < N_LOC
        gi = core * N_LOC + np.minimum(loc, N_LOC - 1)
        sl = slice(core * NSTRIP, (core + 1) * NSTRIP)
        all_vals[:, sl] = np.where(valid, v, -np.inf)
        all_gidx[:, sl] = gi

    # top-k per query; break ties by lowest global index (jax.lax.top_k order)
    m = min(2 * k, all_vals.shape[1] - 1)
    part = np.argpartition(-all_vals, m, axis=1)[:, : m + 1]
    rows = np.arange(B)[:, None]
    pv = all_vals[rows, part]
    pg = all_gidx[rows, part]
    order = np.lexsort((pg, -pv), axis=1)[:, :k]
    out_vals = pv[rows, order]
    out_idx = pg[rows, order]
    out_ids = ids[out_idx]
    return out_vals, out_ids

